# revision 42
# baseline (speedup 1.0000x reference)
"""Trainium2 Bass kernel v3 for the 6-layer post-LN transformer encoder.

Data-parallel over batch: 8 NeuronCores x 2 batches each, weights replicated.
Feature-major activations hT[d, token]; weight-stationary matmuls in f32r
(full rate at free-dim>=256).  Attention internals (Q/K stacks, E, V', ctx,
Wo) and FFN weights run in bf16.

v3 changes vs v2 (trace-driven):
- proj_qk drain re-layout moved off ACT/DVE onto SBUF->SBUF DMAs (sync HW
  DGE queue); psum->bf16 staging is one cheap contiguous cast per mt.
- proj_qk psums moved to 1-bank pb tiles so they never contend with the
  attention sA/sB psum rotation (pa) or the interleaved LN stat psums.
- Softmax denominator reciprocal reads the ctx psum column directly
  (den copy ops removed).
- LN rsqrt computed as exp(-0.5*ln(x)) so every ACT function used lives in
  the one natural_log_exp table -> zero ACT table swaps per layer.
- Vector-engine work split: ctx scatter, LN z-finish, squares, V' mask
  multiplies spread across DVE and Pool (gpsimd).
- LN1(c0) stats interleaved into the Wo(c1) matmul stream.
- Weight DMAs all on the scalar HW DGE queue; re-layout DMAs on sync.
"""

import os
import sys

import numpy as np
import ml_dtypes

for _p in ("/opt/trn_rl_repo", "/root/.axon_site/_ro/trn_rl_repo"):
    if os.path.isdir(_p) and _p not in sys.path:
        sys.path.append(_p)

import concourse.bass as bass
import concourse.mybir as mybir
from concourse import bacc
from concourse.tile import TileContext
from concourse.bass_utils import run_bass_kernel_spmd
from concourse.masks import make_identity

B, S, D, H, Dh, F, L, V = 16, 512, 512, 8, 64, 2048, 6, 32000
NCORES = 8
BPC = B // NCORES          # batches per core
NT = BPC * S               # tokens per core (1024)
DT = D // 128              # 4
FT = F // 128              # 16
NG = BPC * 8               # pseudo attention groups per core (16)
f32 = mybir.dt.float32
f32r = mybir.dt.float32r
bf16 = mybir.dt.bfloat16
AF = mybir.ActivationFunctionType
ALU = mybir.AluOpType
BF = ml_dtypes.bfloat16

_CACHE = {}
_UID = [0]


def _nm(p):
    _UID[0] += 1
    return f"{p}{_UID[0]}"


# MISC column layout: [bo(0:4) | b2(4:8) | gresO(8:12) | gres1(12:16)
#                      | cq(16:20) | ck(20:24) | b1(24:40)]
MISC_W = 40


def _build():
    nc = bacc.Bacc(None, target_bir_lowering=False)

    H0 = nc.dram_tensor("H0", [DT, 128, NT], f32r, kind="ExternalInput")
    VMASK = nc.dram_tensor("VMASK", [128, NG * 4 * 65], bf16, kind="ExternalInput")
    FIN = nc.dram_tensor("FIN", [128, 8], f32, kind="ExternalInput")
    WQ = nc.dram_tensor("WQ", [L, 128, DT * DT * 128], f32r, kind="ExternalInput")
    WK = nc.dram_tensor("WK", [L, 128, DT * DT * 128], f32r, kind="ExternalInput")
    WV = nc.dram_tensor("WV", [L, 128, DT * D], f32r, kind="ExternalInput")
    WO = nc.dram_tensor("WO", [L, 128, DT * DT * 128], bf16, kind="ExternalInput")
    W1 = nc.dram_tensor("W1", [L, 128, DT * F], bf16, kind="ExternalInput")
    W2 = nc.dram_tensor("W2", [L, 128, FT * D], bf16, kind="ExternalInput")
    MISC = nc.dram_tensor("MISC", [L, 128, MISC_W], f32, kind="ExternalInput")
    OUT = nc.dram_tensor("OUT", [NT, D], f32, kind="ExternalOutput")

    from contextlib import ExitStack

    dbg = os.environ.get("KV2_DBG", "")

    with TileContext(nc) as tc:
        with ExitStack() as st:
            cst = st.enter_context(tc.tile_pool(name="cst", bufs=1))
            wbig = st.enter_context(tc.tile_pool(name="wbig", bufs=3))
            wop = st.enter_context(tc.tile_pool(name="wop", bufs=2))
            w1p = st.enter_context(tc.tile_pool(name="w1p", bufs=1))
            w2p = st.enter_context(tc.tile_pool(name="w2p", bufs=1))
            mscp = st.enter_context(tc.tile_pool(name="mscp", bufs=2))
            act = st.enter_context(tc.tile_pool(name="act", bufs=5))
            hpre = st.enter_context(tc.tile_pool(name="hpre", bufs=10))
            ftp = st.enter_context(tc.tile_pool(name="ftp", bufs=4))
            ctxp = st.enter_context(tc.tile_pool(name="ctxp", bufs=1))
            qks = st.enter_context(tc.tile_pool(name="qks", bufs=1))
            vbp = st.enter_context(tc.tile_pool(name="vbp", bufs=1))
            ep = st.enter_context(tc.tile_pool(name="ep", bufs=4))
            uwp = st.enter_context(tc.tile_pool(name="uwp", bufs=3))
            rbp = st.enter_context(tc.tile_pool(name="rbp", bufs=1))
            sml = st.enter_context(tc.tile_pool(name="sml", bufs=6))
            pa = st.enter_context(tc.tile_pool(name="pa", bufs=2, space="PSUM"))
            pb = st.enter_context(tc.tile_pool(name="pb", bufs=4, space="PSUM"))

            # ---- constants ----
            ident = cst.tile([128, 128], f32, tag="identf")
            make_identity(nc, ident[:, :])
            identr = cst.tile([128, 128], f32r, tag="identr")
            nc.vector.tensor_copy(identr[:, :], ident[:, :])
            ones_f = cst.tile([128, 1], f32, tag="onesf")
            nc.gpsimd.memset(ones_f[:, :], 1.0)
            ones_r = cst.tile([128, 1], f32r, tag="onesr")
            nc.vector.tensor_copy(ones_r[:, :], ones_f[:, :])
            vmask_sb = cst.tile([128, NG * 4 * 65], bf16, tag="vmask")
            nc.scalar.dma_start(vmask_sb[:, :], VMASK[:, :])
            fin_sb = cst.tile([128, 8], f32, tag="fin")
            nc.scalar.dma_start(fin_sb[:, :], FIN[:, :])
            eps_sb = cst.tile([1, 1], f32, tag="eps")
            nc.gpsimd.memset(eps_sb[:, :], float(1e-5 * D * D))

            # ---- persistent attention tiles ----
            q2 = qks.tile([128, NG // 2 * 512], bf16, tag="q2")
            k2 = qks.tile([128, NG // 2 * 512], bf16, tag="k2")
            vb = vbp.tile([128, NG * 4 * 65], bf16, tag="vb")
            vb_v = vb.rearrange("p (blk c) -> p blk c", c=65)
            # col 64 = ones; VMASK multiply turns it into the 0/1 mask, so
            # the ctx matmul puts the softmax denominator on psum partition 64.
            nc.gpsimd.memset(vb_v[:, :, 64:65], 1.0)

            def dump1(i, ap):
                """Debug: stage one [128,512] AP as f32 and DMA to OUT rows."""
                stg = hpre.tile([128, 512], f32, tag="osb", bufs=2,
                                name=_nm("dbg"))
                nc.vector.tensor_copy(stg[:, :], ap)
                nc.sync.dma_start(OUT[i * 128:(i + 1) * 128, :], stg[:, :])

            def dump8(aps):
                for i, ap in enumerate(aps[:8]):
                    dump1(i, ap)

            def dumpwide(t):
                """Debug: dump a [128, 4096] tile as 8 blocks."""
                dump8([t[:, i * 512:(i + 1) * 512] for i in range(8)])

            # ---- LN helpers ----
            def emit_sq(h_in):
                """Squares of the 4 feature tiles, spread ACT/DVE/Pool."""
                sqs = []
                for kt in range(DT):
                    sq = hpre.tile([128, 512], f32r, tag="tmp", bufs=5,
                                   name=_nm("sq"))
                    if kt % 2 == 0:
                        nc.scalar.square(sq[:, :], h_in[kt][:, :])
                    else:
                        nc.vector.tensor_mul(sq[:, :], h_in[kt][:, :],
                                             h_in[kt][:, :])
                    sqs.append(sq)
                return sqs

            def emit_stat(pool, tiles):
                """Sum-over-partitions of 4 tiles -> [1,512] psum."""
                stp = pool.tile([1, 512], f32, tag=pool.name, name=_nm("st"))
                for kt in range(DT):
                    nc.tensor.matmul(
                        stp[0:1, :], ones_r[:, :], tiles[kt][:, :],
                        start=(kt == 0), stop=(kt == DT - 1),
                    )
                return stp

            def ln_finish(st0, st1, h_in, z_out, zdt, out_tiles=None,
                          use_pool=False, affine=False):
                """Smalls + broadcasts + normalize: z' = (h-mean)*inv/D.
                invp = exp(-0.5*ln(varp+eps*D^2)) keeps ACT on the ln/exp
                table (no Sqrt -> no table swaps)."""
                sq0 = sml.tile([1, 512], f32, tag="sml", name=_nm("sq0"))
                nc.scalar.square(sq0[:, :], st0[0:1, :])
                varp = sml.tile([1, 512], f32, tag="sml", name=_nm("varp"))
                nc.vector.scalar_tensor_tensor(
                    varp[:, :], st1[0:1, :], float(D), sq0[:, :],
                    op0=ALU.mult, op1=ALU.subtract,
                )
                sdp = sml.tile([1, 512], f32, tag="sml", name=_nm("sdp"))
                nc.scalar.activation(
                    sdp[:, :], varp[:, :], AF.Sqrt, bias=eps_sb[0:1, 0:1],
                    scale=1.0,
                )
                invp = sml.tile([1, 512], f32, tag="sml", name=_nm("invp"))
                nc.vector.reciprocal_approx_fast(invp[:, :], sdp[0:1, :])
                U = uwp.tile([128, 512], f32, tag="uw", name=_nm("U"))
                nc.gpsimd.partition_broadcast(U[:, :], invp[0:1, :])
                b2c = sml.tile([1, 512], f32, tag="sml", name=_nm("b2c"))
                nc.vector.scalar_tensor_tensor(
                    b2c[:, :], st0[0:1, :], float(-1.0 / D), invp[:, :],
                    op0=ALU.mult, op1=ALU.mult,
                )
                B2 = uwp.tile([128, 512], f32, tag="uw", name=_nm("B2"))
                nc.gpsimd.partition_broadcast(B2[:, :], b2c[0:1, :])
                t1s = []
                for kt in range(DT):
                    eng = nc.gpsimd if (use_pool and kt >= 2) else nc.vector
                    t1 = hpre.tile([128, 512], f32r, tag="tmp", bufs=5,
                                   name=_nm("t1"))
                    eng.tensor_mul(t1[:, :], h_in[kt][:, :], U[:, :])
                    t1s.append(t1)
                b2ks = None
                if affine:
                    b2ks = []
                    for kt in range(DT):
                        b2k = hpre.tile([128, 512], f32, tag="tmp", bufs=5,
                                        name=_nm("b2k"))
                        nc.vector.tensor_scalar(
                            b2k[:, :], B2[:, :], fin_sb[:, kt:kt + 1],
                            fin_sb[:, 4 + kt:4 + kt + 1], ALU.mult, ALU.add)
                        b2ks.append(b2k)
                for kt in range(DT):
                    eng = nc.gpsimd if (use_pool and kt >= 2) else nc.vector
                    if affine:
                        nc.vector.scalar_tensor_tensor(
                            out_tiles[kt], t1s[kt][:, :],
                            fin_sb[:, kt:kt + 1], b2ks[kt][:, :],
                            op0=ALU.mult, op1=ALU.add)
                    elif out_tiles is not None:
                        eng.tensor_add(out_tiles[kt], t1s[kt][:, :], B2[:, :])
                    else:
                        z = hpre.tile([128, 512], zdt, tag="hpz", bufs=10,
                                      name=_nm("z"))
                        eng.tensor_add(z[:, :], t1s[kt][:, :], B2[:, :])
                        z_out[kt] = z

            # ---- h0 (host-gathered embedding + positional encoding) ----
            hT = []
            for dt in range(DT):
                t = act.tile([128, NT], f32r, tag="act", name=_nm("h0"))
                nc.scalar.dma_start(t[:, :], H0[dt, :, :])
                hT.append(t)
            if dbg == "h0":
                dump8([hT[dt][:, c * 512:(c + 1) * 512]
                       for dt in range(DT) for c in range(2)])

            # stack layout: S[ch*64 + d, g2*512 + h*64 + sm]
            #   = proj[h*64+d, (g2*64+sm) of chunk ch];  group g = ch*8+g2
            q2_hv = q2.rearrange("p (g2 h sm) -> p h g2 sm", h=8, sm=64)
            k2_hv = k2.rearrange("p (g2 h sm) -> p h g2 sm", h=8, sm=64)

            pend = None
            pend_sq = None
            for l in range(L):
                # ---- weight loads (all on the scalar HW DGE queue) ----
                w1_sb = w1p.tile([128, DT * F], bf16, tag="w1")
                nc.sync.dma_start(w1_sb[:, 0:2048], W1[l, :, 0:2048])
                nc.scalar.dma_start(w1_sb[:, 2048:4096], W1[l, :, 2048:4096])
                wv_sb = wbig.tile([128, 2048], f32r, tag="wbig", name=_nm("wv"))
                nc.scalar.dma_start(wv_sb[:, :], WV[l, :, :])
                msc = mscp.tile([128, MISC_W], f32, tag="msc")
                nc.scalar.dma_start(msc[:, :], MISC[l, :, :])
                wq_sb = wbig.tile([128, 2048], f32r, tag="wbig", name=_nm("wq"))
                nc.sync.dma_start(wq_sb[:, :], WQ[l, :, :])
                wk_sb = wbig.tile([128, 2048], f32r, tag="wbig", name=_nm("wk"))
                nc.sync.dma_start(wk_sb[:, :], WK[l, :, :])
                nc.sync.dma_start(w1_sb[:, 4096:6144], W1[l, :, 4096:6144])
                nc.scalar.dma_start(w1_sb[:, 6144:8192], W1[l, :, 6144:8192])
                wo_sb = wop.tile([128, 2048], bf16, tag="wo")
                nc.scalar.dma_start(wo_sb[:, :], WO[l, :, :])

                # ---- per-chunk Q/K projection -> packed stacks ----
                # drains go straight psum -> strided bf16 stack slices, one
                # [64,512] op per (mt, hh): hh=0 on ACT (+bias), hh=1 on DVE.
                def proj_qk(W_sb, st_hv, ccol, q_on_act, ch):
                    for mt in range(DT):
                        ps = pb.tile([128, 512], f32, tag="pb", name=_nm("pqk"))
                        for kt in range(DT):
                            nc.tensor.matmul(
                                ps[:, :],
                                W_sb[:, kt * 512 + mt * 128:
                                     kt * 512 + mt * 128 + 128],
                                hT[kt][:, ch * 512:(ch + 1) * 512],
                                start=(kt == 0),
                                stop=(kt == DT - 1),
                            )
                        ps_v = ps.rearrange("p (g2 sm) -> p g2 sm", sm=64)
                        for hh in range(2):
                            dst = st_hv[ch * 64:ch * 64 + 64, 2 * mt + hh, :, :]
                            src = ps_v[hh * 64:hh * 64 + 64, :, :]
                            col = msc[hh * 64:hh * 64 + 64,
                                      ccol + mt:ccol + mt + 1]
                            if hh == 0:
                                nc.scalar.activation(
                                    dst, src, AF.Identity, bias=col, scale=1.0,
                                )
                            else:
                                nc.vector.tensor_scalar(
                                    dst, src, col, None, ALU.add,
                                )

                # ---- V projection (per tt-pair) -> vb blocks + mask mult ----
                def proj_v(tt_lo, tt_hi):
                    for tt in range(tt_lo, tt_hi):
                        ps = pb.tile([128, 512], f32, tag="pb", name=_nm("pv"))
                        for kt in range(DT):
                            nc.tensor.matmul(
                                ps[:, :],
                                hT[kt][:, tt * 128:(tt + 1) * 128],
                                wv_sb[:, kt * 512:(kt + 1) * 512],
                                start=(kt == 0),
                                stop=(kt == DT - 1),
                            )
                        ps_v = ps.rearrange("p (k2 pi c) -> p k2 pi c",
                                            pi=2, c=64)
                        for gam in range(2):
                            g = 2 * tt + gam
                            for pi in range(2):
                                if gam == 0:
                                    nc.scalar.copy(
                                        vb_v[pi * 64:pi * 64 + 64,
                                             g * 4:g * 4 + 4, 0:64],
                                        ps_v[gam * 64:gam * 64 + 64, :, pi, :],
                                    )
                                else:
                                    nc.vector.tensor_copy(
                                        vb_v[pi * 64:pi * 64 + 64,
                                             g * 4:g * 4 + 4, 0:64],
                                        ps_v[gam * 64:gam * 64 + 64, :, pi, :],
                                    )
                    c0 = tt_lo * 2 * 4 * 65
                    c1 = tt_hi * 2 * 4 * 65
                    nc.vector.tensor_mul(
                        vb[:, c0:c1], vb[:, c0:c1], vmask_sb[:, c0:c1])

                # ---- QK(c0); finish pending LN2(c1) of the previous layer
                #      under these matmuls; V(c0); attention pipeline with
                #      chunk-1 projections inserted mid-stream ----
                proj_qk(wq_sb, q2_hv, 16, True, 0)
                if pend is not None:
                    p_hin, p_outs = pend
                    p_st0 = emit_stat(pa, p_hin)
                    p_st1 = emit_stat(pa, pend_sq)
                proj_qk(wk_sb, k2_hv, 20, False, 0)
                if pend is not None:
                    ln_finish(p_st0, p_st1, p_hin, None, f32r,
                              out_tiles=p_outs)
                    pend = None
                dmye = cst.tile([1, 1], f32, tag="dmye", name=_nm("dmy"))
                nc.scalar.activation(dmye[:, :], eps_sb[0:1, 0:1], AF.Exp)
                proj_v(0, 2)

                # W2 issued here: its buffer gate (prev layer readers) is
                # already open, so the descriptors never block the queue head
                w2_sb = w2p.tile([128, FT * D], bf16, tag="w2")
                nc.sync.dma_start(w2_sb[:, 0:4096], W2[l, :, 0:4096])
                nc.scalar.dma_start(w2_sb[:, 4096:8192], W2[l, :, 4096:8192])

                ctxT = ctxp.tile([128, DT * NT], bf16, tag="ctx")
                ctx_v = ctxT.rearrange("p (dt t) -> p dt t", t=NT)

                # ---- Wo chain for one (chunk, mt): psum -> bias -> residual
                h_pre = [[None] * DT for _ in range(2)]
                z1 = [[None] * DT for _ in range(2)]

                def wo_mt(ch, mt):
                    ps = pb.tile([128, 512], f32, tag="pb", name=_nm("pwo"))
                    for kt in range(DT):
                        nc.tensor.matmul(
                            ps[:, :],
                            wo_sb[:, kt * 512 + mt * 128:
                                  kt * 512 + mt * 128 + 128],
                            ctx_v[:, kt, ch * 512:(ch + 1) * 512],
                            start=(kt == 0),
                            stop=(kt == DT - 1),
                        )
                    ot = hpre.tile([128, 512], f32r, tag="tmp", bufs=5,
                                   name=_nm("ot"))
                    nc.scalar.activation(
                        ot[:, :], ps[:, :], AF.Identity,
                        bias=msc[:, mt:mt + 1], scale=1.0,
                    )
                    hp = hpre.tile([128, 512], f32r, tag="hl", name=_nm("hpre"))
                    nc.vector.scalar_tensor_tensor(
                        hp[:, :],
                        hT[mt][:, ch * 512:(ch + 1) * 512],
                        msc[:, 8 + mt:8 + mt + 1],
                        ot[:, :],
                        op0=ALU.mult,
                        op1=ALU.add,
                    )
                    h_pre[ch][mt] = hp

                eAs, eBs = {}, {}
                sq_c0 = st0_c0 = st1_c0 = None
                for gi in range(NG + 1):
                    if gi == 1:
                        proj_v(2, 4)
                        if l == 0 and dbg == "vb":
                            dumpwide(vb)
                    if gi == 4:
                        proj_qk(wq_sb, q2_hv, 16, True, 1)
                        proj_qk(wk_sb, k2_hv, 20, False, 1)
                        if l == 0 and dbg == "q2":
                            dumpwide(q2)
                        if l == 0 and dbg == "k2":
                            dumpwide(k2)
                    if gi == 5:
                        proj_v(4, 6)
                    if gi == 6:
                        proj_v(6, 8)
                    # Wo(c0) + LN1(c0) ride the chunk-1 half of the pipeline:
                    # groups 0..7 (chunk 0) are fully scattered by gi==9.
                    if gi == 13:
                        wo_mt(0, 0)
                        wo_mt(0, 1)
                    if gi == 14:
                        wo_mt(0, 2)
                        wo_mt(0, 3)
                        sq_c0 = emit_sq(h_pre[0])
                    if gi == 15:
                        st0_c0 = emit_stat(pb, h_pre[0])
                        st1_c0 = emit_stat(pb, sq_c0)
                    if gi < NG:
                        g = gi
                        gch = g // 8
                        g2 = g % 8
                        sA = pa.tile([128, 1024], f32, tag="pa", name=_nm("sA"))
                        sB = pa.tile([128, 1024], f32, tag="pa", name=_nm("sB"))
                        for kt2 in range(4):
                            tgt = sA if kt2 < 2 else sB
                            nc.tensor.matmul(
                                tgt[:, (kt2 % 2) * 512:(kt2 % 2) * 512 + 512],
                                k2[gch * 64:gch * 64 + 64,
                                   g2 * 512 + kt2 * 128:g2 * 512 + kt2 * 128 + 128],
                                q2[gch * 64:gch * 64 + 64, g2 * 512:g2 * 512 + 512],
                                start=True,
                                stop=True,
                            )
                        eA = ep.tile([128, 1024], bf16, tag="ep", name=_nm("eA"))
                        nc.scalar.activation(eA[:, :], sA[:, :], AF.Exp)
                        eB = ep.tile([128, 1024], bf16, tag="ep", name=_nm("eB"))
                        nc.scalar.activation(eB[:, :], sB[:, :], AF.Exp)
                        eAs[g], eBs[g] = eA, eB
                        if l == 0 and dbg == "eg01" and g < 2:
                            for i, ap in enumerate(
                                    [eA[:, 0:512], eA[:, 512:1024],
                                     eB[:, 0:512], eB[:, 512:1024]]):
                                dump1(g * 4 + i, ap)
                    if gi > 0:
                        g = gi - 1
                        eA, eB = eAs.pop(g), eBs.pop(g)
                        cps = pb.tile([128, 512], f32, tag="pb", name=_nm("cps"))
                        for kt2 in range(4):
                            src = eA if kt2 < 2 else eB
                            nc.tensor.matmul(
                                cps[0:65, :],
                                vb_v[:, g * 4 + kt2, :],
                                src[:, (kt2 % 2) * 512:(kt2 % 2) * 512 + 512],
                                start=(kt2 == 0),
                                stop=(kt2 == 3),
                            )
                    if gi > 0:
                        g = gi - 1
                        if l == 0 and dbg == "cps" and g < 4:
                            dump1(g, cps[:, :])
                        den = sml.tile([1, 512], f32, tag="sml", name=_nm("den"))
                        nc.vector.tensor_copy(den[:, :], cps[64:65, :])
                        rec = sml.tile([1, 512], f32, tag="sml", name=_nm("rec"))
                        nc.vector.reciprocal_approx_fast(rec[:, :], den[0:1, :])
                        rb = rbp.tile([64, 512], f32, tag="rb")
                        nc.gpsimd.partition_broadcast(rb[:, :], rec[0:1, :])
                        # ctx scatter: query col = hq*64+sq, hq = 2*dt+pi
                        cps_v = cps.rearrange("p (dt pi sq) -> p dt pi sq",
                                              pi=2, sq=64)
                        rb_v = rb.rearrange("p (dt pi sq) -> p dt pi sq",
                                            pi=2, sq=64)
                        tb = g * 64
                        for pi in range(2):
                            eng = nc.vector
                            eng.tensor_mul(
                                ctx_v[pi * 64:pi * 64 + 64, :, tb:tb + 64],
                                cps_v[0:64, :, pi, :],
                                rb_v[:, :, pi, :],
                            )

                if l == 0 and dbg == "ctx":
                    dump8([ctx_v[:, dt, c * 512:(c + 1) * 512]
                           for dt in range(DT) for c in range(2)])

                # LN1(c0) finish after all exps (keeps the ACT exp table
                # resident through the whole attention stream); wo(c1)
                # provides PE cover for the finish chain
                ln_finish(st0_c0, st1_c0, h_pre[0], z1[0], bf16)
                for mt in range(DT):
                    wo_mt(1, mt)

                # ---- FFN; LN1(c1) / LN2(c0) stats interleave into the
                #      matmul stream ----
                hT_next = [act.tile([128, NT], f32r, tag="act", name=_nm("hT"))
                           for _ in range(DT)]
                h_pre2 = [[None] * DT for _ in range(2)]
                for ch in range(2):
                    if ch == 0:
                        ins_in = h_pre[1]
                        sq_at, st0_at, st1_at = 2, 2, 3
                    else:
                        ins_in = h_pre2[0]
                        sq_at, st0_at, st1_at = 1, 1, 2
                    o2ps = None
                    for mtp in range(FT // 2):
                        if mtp == sq_at:
                            ins_sq = emit_sq(ins_in)
                        if mtp == st0_at:
                            st0_i = emit_stat(pb, ins_in)
                        if mtp == st1_at:
                            st1_i = emit_stat(pb, ins_sq)
                            if ch == 0:
                                ln_finish(st0_i, st1_i, ins_in, z1[1], bf16)
                            else:
                                ln_finish(st0_i, st1_i, ins_in, None, f32r,
                                          out_tiles=[t[:, 0:512]
                                                     for t in hT_next],
                                          affine=(l == L - 1))
                        fts = []
                        for half in range(2):
                            mtf = 2 * mtp + half
                            ps = pb.tile([128, 512], f32, tag="pb",
                                         name=_nm("pw1"))
                            for kt in range(DT):
                                nc.tensor.matmul(
                                    ps[:, :],
                                    w1_sb[:, mtf * 512 + kt * 128:
                                          mtf * 512 + kt * 128 + 128],
                                    z1[ch][kt][:, :],
                                    start=(kt == 0),
                                    stop=(kt == DT - 1),
                                )
                            f_t = ftp.tile([128, 512], bf16, tag="ft",
                                           name=_nm("ft"))
                            nc.scalar.activation(
                                f_t[:, :], ps[:, :],
                                AF.Relu, bias=msc[:, 24 + mtf:24 + mtf + 1],
                                scale=1.0,
                            )
                            fts.append((mtf, f_t))
                        if o2ps is None:
                            o2ps = [pa.tile([128, 1024], f32, tag="pa",
                                            name=_nm("o2"))
                                    for _ in range(2)]
                        for mtf, f_t in fts:
                            for mtd in range(DT):
                                nc.tensor.matmul(
                                    o2ps[mtd // 2][:, (mtd % 2) * 512:
                                                   (mtd % 2) * 512 + 512],
                                    w2_sb[:, mtf * 512 + mtd * 128:
                                          mtf * 512 + mtd * 128 + 128],
                                    f_t[:, :],
                                    start=(mtf == 0),
                                    stop=(mtf == FT - 1),
                                )
                    for mtd in range(DT):
                        ft2 = hpre.tile([128, 512], f32r, tag="tmp", bufs=5,
                                        name=_nm("f2"))
                        nc.scalar.activation(
                            ft2[:, :],
                            o2ps[mtd // 2][:, (mtd % 2) * 512:
                                           (mtd % 2) * 512 + 512],
                            AF.Identity,
                            bias=msc[:, 4 + mtd:4 + mtd + 1], scale=1.0,
                        )
                        hp2 = hpre.tile([128, 512], f32r, tag="hl",
                                        name=_nm("hp2"))
                        nc.vector.scalar_tensor_tensor(
                            hp2[:, :],
                            z1[ch][mtd][:, :],
                            msc[:, 12 + mtd:12 + mtd + 1],
                            ft2[:, :],
                            op0=ALU.mult,
                            op1=ALU.add,
                        )
                        h_pre2[ch][mtd] = hp2
                if l == 0 and dbg == "hpre":
                    dump8([h_pre[ch][mt][:, :]
                           for mt in range(DT) for ch in range(2)])
                if l == 0 and dbg == "z1":
                    dump8([z1[ch][mt][:, :]
                           for mt in range(DT) for ch in range(2)])

                # LN2(c1) is deferred: squares here, stats+finish ride the
                # next layer's QK(c0) projections (or the epilogue)
                pend = (h_pre2[1], [t[:, 512:1024] for t in hT_next])
                pend_sq = emit_sq(h_pre2[1])
                hT = hT_next

            # epilogue: finish the last layer's LN2(c1)
            p_hin, p_outs = pend
            p_st0 = emit_stat(pa, p_hin)
            p_st1 = emit_stat(pa, pend_sq)

            def store_c(c):
                o_sb = hpre.tile([128, 512], f32, tag="osb", bufs=2,
                                 name=_nm("osb"))
                for dt in range(DT):
                    pt = pb.tile([128, 128], f32r, tag="pb", name=_nm("pt"))
                    nc.tensor.transpose(
                        pt[:, :], hT[dt][:, c * 128:(c + 1) * 128], identr[:, :]
                    )
                    if dt % 2 == 0:
                        nc.vector.tensor_copy(
                            o_sb[:, dt * 128:(dt + 1) * 128], pt[:, :])
                    else:
                        nc.scalar.copy(
                            o_sb[:, dt * 128:(dt + 1) * 128], pt[:, :])
                nc.sync.dma_start(OUT[c * 128:(c + 1) * 128, :], o_sb[:, :])

            if not dbg:
                for c in range(4):
                    store_c(c)
            ln_finish(p_st0, p_st1, p_hin, None, f32r, out_tiles=p_outs,
                      use_pool=True, affine=True)
            if not dbg:
                for c in range(4, 8):
                    store_c(c)

    nc.compile()
    return nc


def _fold_params(Wq, Wk, Wv, Wo, bo, ln1_g, ln1_b, W1, b1, W2, b2,
                 ln2_g, ln2_b):
    """Fold LN affines into neighboring weights (see module docstring)."""
    fz = np.float32
    scale = fz(1.0) / np.sqrt(fz(Dh))
    WqE = np.empty((L, D, D), fz)
    WkE = np.empty((L, D, D), fz)
    WvE = np.empty((L, D, D), fz)
    cq = np.zeros((L, D), fz)
    ck = np.zeros((L, D), fz)
    boE = np.empty((L, D), fz)
    gresO = np.empty((L, D), fz)
    W1E = np.empty((L, D, F), fz)
    b1E = np.empty((L, F), fz)
    b2E = np.empty((L, D), fz)
    gres1 = np.empty((L, D), fz)
    for l in range(L):
        if l == 0:
            g_in = np.ones(D, fz)
            b_in = np.zeros(D, fz)
        else:
            g_in = (ln2_g[l - 1] * fz(D)).astype(fz)
            b_in = ln2_b[l - 1].astype(fz)
        WqE[l] = (g_in[:, None] * Wq[l]) * scale
        cq[l] = (b_in @ Wq[l]) * scale
        WkE[l] = g_in[:, None] * Wk[l]
        ck[l] = b_in @ Wk[l]
        WvE[l] = g_in[:, None] * Wv[l]
        cv = b_in @ Wv[l]
        boE[l] = bo[l] + cv @ Wo[l] + b_in
        gresO[l] = g_in
        W1E[l] = (ln1_g[l] * fz(D))[:, None] * W1[l]
        b1E[l] = b1[l] + ln1_b[l] @ W1[l]
        b2E[l] = b2[l] + ln1_b[l]
        gres1[l] = ln1_g[l] * fz(D)
    gfin = (ln2_g[L - 1] * fz(D)).astype(fz)
    bfin = ln2_b[L - 1].astype(fz)
    return dict(WqE=WqE, WkE=WkE, WvE=WvE, cq=cq, ck=ck, boE=boE,
                gresO=gresO, W1E=W1E, b1E=b1E, b2E=b2E, gres1=gres1,
                gfin=gfin, bfin=bfin)


def _host_prep(x, batch_length, embed, Wq, Wk, Wv, Wo, bo, ln1_g, ln1_b,
               W1, b1, W2, b2, ln2_g, ln2_b):
    fz = np.float32
    x = np.asarray(x)
    bl = np.asarray(batch_length)
    g = lambda a: np.asarray(a, fz)
    embed = g(embed)
    P = _fold_params(g(Wq), g(Wk), g(Wv), g(Wo), g(bo), g(ln1_g), g(ln1_b),
                     g(W1), g(b1), g(W2), g(b2), g(ln2_g), g(ln2_b))

    def tile_kmt(w):  # [L, D, D] -> [L, 128, DT*DT*128], col kt*512+mt*128+c
        return np.ascontiguousarray(
            w.reshape(L, DT, 128, DT, 128).transpose(0, 2, 1, 3, 4)
            .reshape(L, 128, DT * DT * 128))

    WQc = tile_kmt(P["WqE"])
    WKc = tile_kmt(P["WkE"])
    WVc = np.ascontiguousarray(
        P["WvE"].reshape(L, DT, 128, D).transpose(0, 2, 1, 3)
        .reshape(L, 128, DT * D))
    WOc = tile_kmt(g(Wo)).astype(BF)
    W1c = np.ascontiguousarray(
        P["W1E"].reshape(L, DT, 128, FT, 128).transpose(0, 2, 3, 1, 4)
        .reshape(L, 128, DT * F)).astype(BF)
    W2c = np.ascontiguousarray(
        g(W2).reshape(L, FT, 128, DT, 128).transpose(0, 2, 1, 3, 4)
        .reshape(L, 128, FT * D)).astype(BF)

    tile_c = lambda v, nt: np.ascontiguousarray(
        v.reshape(L, nt, 128).transpose(0, 2, 1))
    misc = np.zeros((L, 128, MISC_W), fz)
    misc[:, :, 0:4] = tile_c(P["boE"], DT)
    misc[:, :, 4:8] = tile_c(P["b2E"], DT)
    misc[:, :, 8:12] = tile_c(P["gresO"], DT)
    misc[:, :, 12:16] = tile_c(P["gres1"], DT)
    misc[:, :, 16:20] = tile_c(P["cq"], DT)
    misc[:, :, 20:24] = tile_c(P["ck"], DT)
    misc[:, :, 24:40] = tile_c(P["b1E"], FT)

    fin = np.zeros((128, 8), fz)
    fin[:, 0:4] = P["gfin"].reshape(DT, 128).T
    fin[:, 4:8] = P["bfin"].reshape(DT, 128).T

    # VMASK: 0/1 by pseudo-position padding of mask batch m = local group g
    pad = (x == 0)
    vmask = np.zeros((128, NG * 4 * 65), fz)
    p = np.arange(128)
    for gg in range(NG):
        for kt2 in range(4):
            jk = (p % 64) * 8 + 2 * kt2 + p // 64
            m01 = np.where(pad[gg, jk], fz(0), fz(1))
            b0 = (gg * 4 + kt2) * 65
            vmask[:, b0:b0 + 65] = m01[:, None]
    vmask = vmask.astype(BF)

    # sinusoidal PE, exact float32 replication of the reference formula
    pos = np.arange(S, dtype=fz)[:, None]
    i = np.arange(D, dtype=fz)[None, :]
    ang = (pos / np.power(fz(10000.0), (fz(2.0) * i / fz(D)),
                          dtype=fz)).astype(fz)
    pe = ang.copy()
    pe[:, 0::2] = np.sin(ang[:, 0::2])
    pe[:, 1::2] = np.cos(ang[:, 1::2])

    in_maps = []
    for c in range(NCORES):
        bsel = slice(BPC * c, BPC * c + BPC)
        lm = (np.arange(S)[None, :] < bl[bsel, None]).astype(fz)
        h0 = embed[x[bsel]] + pe[None, :, :] * lm[:, :, None]  # [BPC, S, D]
        h0t = np.ascontiguousarray(
            h0.transpose(2, 0, 1).reshape(D, NT).reshape(DT, 128, NT))
        in_maps.append({
            "H0": h0t, "VMASK": vmask, "FIN": fin,
            "WQ": WQc, "WK": WKc, "WV": WVc, "WO": WOc,
            "W1": W1c, "W2": W2c, "MISC": misc,
        })
    return in_maps


def kernel(**inputs):
    if "nc" not in _CACHE:
        _CACHE["nc"] = _build()
    nc = _CACHE["nc"]
    in_maps = _host_prep(**inputs)
    res = None
    for attempt in range(3):
        try:
            res = run_bass_kernel_spmd(nc, in_maps, core_ids=list(range(NCORES)))
            break
        except Exception:
            if attempt == 2:
                raise
    out = np.empty((B, S, D), np.float32)
    for c in range(NCORES):
        out[BPC * c:BPC * c + BPC] = res.results[c]["OUT"].reshape(BPC, S, D)
    return out
